# revision 28
# baseline (speedup 1.0000x reference)
"""DeepSeekV2-style MLA attention forward on 8 Trainium2 NeuronCores.

Sharding: 2-way data-parallel over batch x 4-way tensor-parallel over heads
(4 heads per core). The shared low-rank q_a/kv_a projections are sharded
over TOKENS within each batch's TP group; o_proj partial outputs are summed
on the host (TP unshard).

Collective pipelining: the gathers move UNNORMALIZED bias-added latents in
bf16 plus a sum-of-squares row, chunked so they can be issued while the
projection is still running (ckv first, then two q_a chunks). Each
consumer core recomputes rstd from the gathered sumsq and scales the
gathered rhs tiles; the rmsnorm gammas are folded into the q_b/kv_b
weights on the host. This removes the "normalize -> gather -> project"
serialization and hides most of the collective latency.

PE discipline: every hot matmul is K=128/M=128/N=512 with the same
geometry — k_rot is zero-padded to 128 contraction rows (q_rope padding
rows are zero-initialized), and the softmax denominator / rmsnorm sumsq
reductions use a [128,128] ones stationary so the result arrives
pre-broadcast across partitions. Mixed-geometry neighbors were measured to
cost ~270ns extra per matmul on TRN2.

Attention: scores are computed transposed s^T[k, q], exp'd on ScalarE
without max-subtraction (scores are provably small), masked on the causal
diagonal blocks; AV uses p^T as the moving operand. q^T/k_nope/v/k_rot
stay SBUF-resident between phases; phase-B weights are loaded once.
Everything below f32 PSUM accumulation is bf16.
"""
import math
import sys

import numpy as np

try:
    import concourse.bass as bass  # noqa: F401
except ImportError:  # pragma: no cover
    sys.path.insert(0, "/opt/trn_rl_repo")

import concourse.bass as bass
import concourse.tile as tile
from concourse import bacc, mybir
from concourse.bass_utils import run_bass_kernel_spmd

# ---- problem dims (hardcoded per contest contract) ----
B, S, HID = 2, 2048, 2048
NH = 16
DN, DR, DV = 128, 64, 128
QD = DN + DR                       # 192
QLR, KVLR = 1536, 512
EPS = 1e-6
ROPE_BASE = 10000.0
SCALE = 1.0 / math.sqrt(QD)

N_CORES = 8
TPG = 4                            # TP group size (cores per batch)
HPC = NH // TPG                    # heads per core = 4

F32 = mybir.dt.float32
F32R = mybir.dt.float32r
BF16 = mybir.dt.bfloat16
I32 = mybir.dt.int32
NP_BF16 = mybir.dt.np(mybir.dt.bfloat16)

NKV = KVLR + DR                    # 576 kv_a rows
T_TILE = 512                       # token tile (free dim)
NT = S // T_TILE                   # 4 token tiles
KB = S // 128                      # 16 key tiles of 128

NFO_KV = KVLR // 128               # 4
NFO_QA = QLR // 128                # 12
NHI = HID // 128                   # 16

NGR = NKV + 1                      # 577 gathered ckv rows (+sumsq)
NGQ = QLR + 1                      # 1537 gathered qa rows (+sumsq)
QCH = 768                          # qa gather chunk (2 chunks)

TWO_PI = 2.0 * math.pi
MAGIC = np.float32(1.5 * 2**23)    # round-to-nearest-int magic constant

REPLICA_GROUPS = [[0, 1, 2, 3], [4, 5, 6, 7]]


def _cody_waite_consts():
    def trunc12(x):
        return np.frombuffer(
            (np.frombuffer(np.float32(x).tobytes(), np.uint32)
             & np.uint32(0xFFFFF000)).tobytes(), np.float32)[0]
    c1 = trunc12(np.float64(TWO_PI))
    c2 = trunc12(np.float64(TWO_PI) - np.float64(c1))
    c3 = np.float32(np.float64(TWO_PI) - np.float64(c1) - np.float64(c2))
    return float(c1), float(c2), float(c3)


CW1, CW2, CW3 = _cody_waite_consts()

_BUILD_CACHE = {}


def build_kernel(debug=False):
    key = bool(debug)
    if key in _BUILD_CACHE:
        return _BUILD_CACHE[key]

    nc = bacc.Bacc("TRN2", target_bir_lowering=False, debug=False,
                   num_devices=N_CORES)

    def din(name, shape, dt=BF16):
        return nc.dram_tensor(name, list(shape), dt, kind="ExternalInput").ap()

    # ---- per-core external inputs ----
    # weight/activation inputs are host-swizzled to partition-major layouts
    # so every DMA reads multi-KB contiguous runs per partition
    xTl = din("xTl", [128, NHI * T_TILE])          # [p][hi][s]
    w_qaT = din("w_qaT", [128, NFO_QA * NHI * 128])    # [p][fo][hi][c]
    w_kvaT = din("w_kvaT", [128, 5 * NHI * 128])       # [p][fo][hi][c] padded
    w_qbT = din("w_qbT", [128, NFO_QA * HPC * QD])     # [p][fi][f] ln-folded
    w_kvb_nT = din("w_kvb_nT", [128, NFO_KV * HPC * DN])
    w_kvb_vT = din("w_kvb_vT", [128, NFO_KV * HPC * DV])
    w_oT = din("w_oT", [128, HPC * HID])               # [p][fs][hid]
    b_qa = din("b_qa", [128, NFO_QA], F32)
    b_kva = din("b_kva", [128, 5], F32)            # 576 padded to 640
    pos = din("pos", [1, S], I32)                  # full positions (for q rope)
    pos_l = din("pos_l", [1, T_TILE], I32)         # local positions (k_pe rope)
    inv_freq = din("inv_freq", [128, 1], F32)      # rope inv freqs, 4x repeated
    p128 = din("p128", [128, 128], F32R)           # blockdiag(rotT, rotT)
    ones_sq = din("ones_sq", [128, 128], F32R)     # sumsq reduction+broadcast
    ones_sqb = din("ones_sqb", [128, 128])         # bf16 ones (softmax l)
    masks = din("masks", [128, 4 * T_TILE])        # bf16 causal diag masks

    out = nc.dram_tensor("out", [S, HID], F32, kind="ExternalOutput").ap()

    # collective in/out tensors must stay Internal (cannot be IO); one qa
    # gather (the CC core serializes collectives, so chunking buys nothing)
    qa_sh = nc.dram_tensor("qa_sh", [NGQ, T_TILE], BF16).ap()
    qa_all = nc.dram_tensor("qa_all", [NT, NGQ, T_TILE], BF16).ap()
    ckv_sh = nc.dram_tensor("ckv_sh", [NGR, T_TILE], BF16).ap()
    ckv_all = nc.dram_tensor("ckv_all", [NT, NGR, T_TILE], BF16).ap()

    with tile.TileContext(nc) as tc:
        with tc.tile_pool(name="const", bufs=1) as constp:
            bqa_t = constp.tile([128, NFO_QA], F32)
            nc.gpsimd.dma_start(bqa_t[:], b_qa[:])
            bkva_t = constp.tile([128, 5], F32)
            nc.gpsimd.dma_start(bkva_t[:], b_kva[:])
            ones_t = constp.tile([128, 128], F32R)
            nc.gpsimd.dma_start(ones_t[:], ones_sq[:])
            onesb_t = constp.tile([128, 128], BF16)
            nc.gpsimd.dma_start(onesb_t[:], ones_sqb[:])
            p128_t = constp.tile([128, 128], F32R)
            nc.gpsimd.dma_start(p128_t[:], p128[:])
            ivf_t = constp.tile([128, 1], F32)
            nc.gpsimd.dma_start(ivf_t[:], inv_freq[:])

            def rope_tables(pos_ap, n, cos_dst, sin_dst, rp, tag):
                """Build cos/sin [128, n] tables from int32 positions [1, n]."""
                pos_i = rp.tile([1, n], I32, name=f"pos_i_{tag}")
                nc.gpsimd.dma_start(pos_i[:], pos_ap[:])
                pos_f = rp.tile([1, n], F32, name=f"pos_f_{tag}")
                nc.vector.tensor_copy(pos_f[:], pos_i[:])
                pos_b = rp.tile([128, n], F32, name=f"pos_b_{tag}")
                nc.gpsimd.partition_broadcast(pos_b[:], pos_f[:])
                freqs = rp.tile([128, n], F32, name=f"freqs_{tag}")
                nc.vector.tensor_scalar_mul(freqs[:], pos_b[:], ivf_t[:])
                kr = rp.tile([128, n], F32, name=f"kr_{tag}")
                nc.vector.tensor_scalar(kr[:], freqs[:], 1.0 / TWO_PI,
                                        float(MAGIC), mybir.AluOpType.mult,
                                        mybir.AluOpType.add)
                nc.vector.tensor_scalar_sub(kr[:], kr[:], float(MAGIC))
                red = rp.tile([128, n], F32, name=f"red_{tag}")
                nc.vector.cody_waite_cascade(red[:], freqs[:], kr[:],
                                             CW1, CW2, CW3)
                nc.scalar.activation(sin_dst, red[:],
                                     mybir.ActivationFunctionType.Sin)
                redc = rp.tile([128, n], F32, name=f"redc_{tag}")
                nc.vector.add_range_wrap(redc[:], red[:], math.pi / 2.0,
                                         math.pi, TWO_PI)
                nc.scalar.activation(cos_dst, redc[:],
                                     mybir.ActivationFunctionType.Sin)

            # resident phase-B weight tiles; the DMAs are issued later (on
            # the scalar queue, after the gathers are underway) so they do
            # not starve phase A's critical xa/weight loads at t=0
            wres = tc.alloc_tile_pool(name="wres", bufs=1)
            w_kvb_nT_r = w_kvb_nT.rearrange("p (fi f) -> p fi f", fi=NFO_KV)
            w_kvb_vT_r = w_kvb_vT.rearrange("p (fi f) -> p fi f", fi=NFO_KV)
            w_qbT_r = w_qbT.rearrange("p (fi f) -> p fi f", fi=NFO_QA)
            w_oT_r = w_oT.rearrange("p (fs hid) -> p fs hid", fs=HPC)
            wkn_t = wres.tile([128, NFO_KV, HPC * DN], BF16, name="wkn_t")
            wv_t = wres.tile([128, NFO_KV, HPC * DV], BF16, name="wv_t")
            wqb_t = wres.tile([128, NFO_QA, HPC * QD], BF16, name="wqb_t")
            wo_sb = wres.tile([128, HPC, HID], BF16, name="wo_sb")
            masks_t = wres.tile([128, 4, T_TILE], BF16, name="masks_t")

            # ---------- phase A: local-token kv_a / q_a + k rope + gathers --
            xTl_r = xTl.rearrange("p (hi s) -> p hi s", hi=NHI)
            w_qaT_r = w_qaT.rearrange("p (fo hi c) -> p fo hi c", fo=NFO_QA,
                                      hi=NHI)
            w_kvaT_r = w_kvaT.rearrange("p (fo hi c) -> p fo hi c", fo=5,
                                        hi=NHI)
            qa_sh_r = qa_sh[0:QLR, :].rearrange("(f p) s -> p f s", p=128)

            with nc.named_scope("proj_a"), \
                 tc.tile_pool(name="ap_", bufs=1) as ap_, \
                 tc.tile_pool(name="wa", bufs=4) as wap, \
                 tc.tile_pool(name="va", bufs=2) as vap, \
                 tc.tile_pool(name="pa", bufs=3, space="PSUM") as pap, \
                 tc.tile_pool(name="ssp", bufs=1, space="PSUM") as ssp:
                # local rope tables for k_pe
                cos_l = ap_.tile([128, T_TILE], F32)
                sin_l = ap_.tile([128, T_TILE], F32)
                rope_tables(pos_l, T_TILE, cos_l[:], sin_l[:], ap_, "loc")

                xa = ap_.tile([128, NHI, T_TILE], BF16)
                nc.sync.dma_start(xa[:, 0:8, :], xTl_r[:, 0:8, :])
                nc.sync.dma_start(xa[:, 8:NHI, :], xTl_r[:, 8:NHI, :])
                val_qa = ap_.tile([128, NFO_QA, T_TILE], BF16)
                val_kv = ap_.tile([128, 5, T_TILE], BF16)
                ss_qa = ssp.tile([128, T_TILE], F32, name="ss_qa")
                ss_kv = ssp.tile([128, T_TILE], F32, name="ss_kv")

                def proj_tiles(proj, fo_lo, fo_hi):
                    """Projection tiles [fo_lo, fo_hi): matmul+bias+sumsq.
                    The kv weight input is zero-padded to 640 rows so every
                    tile is a full 128 (rows 576..639 are never read)."""
                    nfo = NFO_QA if proj == 0 else 5
                    wsrc = w_qaT_r if proj == 0 else w_kvaT_r
                    bias_t = bqa_t if proj == 0 else bkva_t
                    vdst = val_qa if proj == 0 else val_kv
                    sst = ss_qa if proj == 0 else ss_kv
                    for fo in range(fo_lo, fo_hi):
                        wt = wap.tile([128, NHI, 128], BF16, tag="wt")
                        nc.sync.dma_start(wt[:], wsrc[:, fo, :, :])
                        ps = pap.tile([128, T_TILE], F32, tag="acc")
                        for hi in range(NHI):
                            nc.tensor.matmul(
                                ps[:], wt[:, hi, :], xa[:, hi, :],
                                start=(hi == 0), stop=(hi == NHI - 1))
                        nc.vector.tensor_scalar_add(
                            vdst[:, fo, :], ps[:], bias_t[:, fo:fo + 1])
                        if proj == 0:
                            nc.sync.dma_start(
                                qa_sh_r[:, fo, :], vdst[:, fo, :])
                        elif fo < 4:
                            nc.sync.dma_start(
                                ckv_sh[fo * 128:(fo + 1) * 128, :],
                                vdst[:, fo, :])
                        if not (proj == 1 and fo == 4):
                            sq = vap.tile([128, T_TILE], F32R, tag="sq")
                            nc.vector.tensor_tensor(
                                sq[:], vdst[:, fo, :], vdst[:, fo, :],
                                mybir.AluOpType.mult)
                            nc.tensor.matmul(
                                sst[:], ones_t[:], sq[:],
                                start=(fo == 0),
                                stop=(fo == nfo - 1 - (proj == 1)))

                def ss_row(sst, row):
                    """Emit sumsq (any partition row — all equal) as bf16."""
                    sr = vap.tile([1, T_TILE], BF16, tag="ssr")
                    nc.scalar.activation(sr[:], sst[0:1, :],
                                         mybir.ActivationFunctionType.Copy)
                    nc.sync.dma_start(row, sr[:])

                # kv first so its gather can hide under the q_a projection
                proj_tiles(1, 0, 5)
                ss_row(ss_kv, ckv_sh[KVLR + DR:NGR, :])
                # k_pe rope (local tokens) -> ckv_sh rows 512..576
                kpe = vap.tile([64, T_TILE], F32R, tag="kpe")
                nc.vector.tensor_copy(kpe[:], val_kv[0:64, 4, :])
                rps = pap.tile([64, T_TILE], F32, tag="rotk")
                nc.tensor.matmul(rps[:], p128_t[0:64, 0:64], kpe[:],
                                 start=True, stop=True)
                tmp = vap.tile([64, T_TILE], F32, tag="tmpk")
                nc.vector.tensor_tensor(tmp[:], cos_l[0:64, :], kpe[:],
                                        mybir.AluOpType.mult)
                rot = vap.tile([64, T_TILE], F32, tag="rotk2")
                nc.vector.tensor_tensor(rot[:], sin_l[0:64, :], rps[:],
                                        mybir.AluOpType.mult)
                kro = vap.tile([64, T_TILE], BF16, tag="kro")
                nc.vector.tensor_tensor(kro[:], tmp[:], rot[:],
                                        mybir.AluOpType.add)
                nc.sync.dma_start(ckv_sh[KVLR:KVLR + DR, :], kro[:])
                nc.gpsimd.collective_compute(
                    "AllGather", mybir.AluOpType.bypass,
                    replica_groups=REPLICA_GROUPS,
                    ins=[ckv_sh[:]], outs=[ckv_all[:]])
                nc.scalar.dma_start(wkn_t[:], w_kvb_nT_r[:])
                nc.scalar.dma_start(wv_t[:], w_kvb_vT_r[:])

                proj_tiles(0, 0, 6)
                nc.scalar.dma_start(wqb_t[:], w_qbT_r[:])
                nc.scalar.dma_start(wo_sb[:], w_oT_r[:])
                nc.scalar.dma_start(
                    masks_t[:], masks.rearrange("p (j t) -> p j t", j=4))
                proj_tiles(0, 6, NFO_QA)
                ss_row(ss_qa, qa_sh[QLR:NGQ, :])
                nc.gpsimd.collective_compute(
                    "AllGather", mybir.AluOpType.bypass,
                    replica_groups=REPLICA_GROUPS,
                    ins=[qa_sh[:]], outs=[qa_all[:]])

            # ---------- phase R: full rope cos/sin tables (for q) ----------
            # runs on vector/scalar/gpsimd while the gathers are in flight
            cos_t = constp.tile([128, NT, T_TILE], F32)
            sin_t = constp.tile([128, NT, T_TILE], F32)
            with nc.named_scope("rope_tables"), \
                 tc.tile_pool(name="ropep", bufs=1) as rp:
                rope_tables(pos, S, cos_t.rearrange("p n t -> p (n t)"),
                            sin_t.rearrange("p n t -> p (n t)"), rp, "full")

            # B/C-phase resident tiles (allocated after phase-A pools close)
            res = tc.alloc_tile_pool(name="res", bufs=1)
            kn_sb = res.tile([128, HPC, S], BF16, name="kn_sb")
            vh_sb = res.tile([128, HPC, KB, DV], BF16, name="vh_sb")
            krot_sb = res.tile([128, S], BF16, name="krot_sb")
            qn_sb = res.tile([128, HPC, S], BF16, name="qn_sb")
            qr_sb = res.tile([128, HPC, S], BF16, name="qr_sb")
            # zero-fill so rows 64..127 contribute 0 to K=128 rope matmuls
            nc.vector.memset(krot_sb[64:128, :], 0.0)
            nc.vector.memset(qr_sb[64:128, :, :], 0.0)

            qa_all_r = qa_all[:, 0:QLR, :].rearrange(
                "n (f p) s -> p n f s", p=128)

            def rstd_tile(ss_row_ap, d, pool, pspool, tag):
                """[128,512] rstd from a gathered bf16 sumsq row. The
                partition broadcast is a PE ones outer-product: the gpsimd
                queue is backed up behind the collectives at this point."""
                ssb = pspool.tile([128, T_TILE], F32, tag=f"{tag}b")
                nc.tensor.matmul(ssb[:], ones_t[0:1, :], ss_row_ap,
                                 start=True, stop=True)
                ms = pool.tile([128, T_TILE], F32, tag=f"{tag}m")
                nc.vector.tensor_scalar(
                    ms[:], ssb[:], 1.0 / d, EPS,
                    mybir.AluOpType.mult, mybir.AluOpType.add)
                std = pool.tile([128, T_TILE], F32, tag=f"{tag}s")
                nc.scalar.activation(std[:], ms[:],
                                     mybir.ActivationFunctionType.Sqrt)
                rstd = pool.tile([128, T_TILE], F32, tag=f"{tag}r")
                nc.vector.reciprocal_approx_fast(rstd[:], std[:])
                return rstd

            # ---------- phase B-kv: k_nope / v for all token tiles ----------
            with nc.named_scope("proj_bkv"), \
                 tc.tile_pool(name="rhk", bufs=2) as rhkp, \
                 tc.tile_pool(name="rsk", bufs=2) as rskp, \
                 tc.tile_pool(name="psk", bufs=1, space="PSUM") as pskp, \
                 tc.tile_pool(name="pbk", bufs=3, space="PSUM") as pbkp:
                for t in range(NT):
                    tsl = slice(t * T_TILE, (t + 1) * T_TILE)
                    ckv_rhs = rhkp.tile([128, NFO_KV, T_TILE], BF16,
                                        tag="ckvrhs")
                    nc.sync.dma_start(
                        ckv_rhs[:],
                        ckv_all[t, 0:KVLR, :].rearrange(
                            "(f p) s -> p f s", p=128))
                    nc.scalar.dma_start(
                        krot_sb[0:64, tsl], ckv_all[t, KVLR:KVLR + DR, :])
                    ssb16 = rskp.tile([1, T_TILE], BF16, tag="ssb16")
                    nc.sync.dma_start(ssb16[:], ckv_all[t, NKV:NGR, :])
                    ssr = rskp.tile([1, T_TILE], F32R, tag="ssr")
                    nc.vector.tensor_copy(ssr[:], ssb16[:])
                    rstd = rstd_tile(ssr[:], KVLR, rskp, pskp, "rk")
                    for f in range(NFO_KV):
                        nc.vector.tensor_tensor(
                            ckv_rhs[:, f, :], ckv_rhs[:, f, :], rstd[:],
                            mybir.AluOpType.mult)
                    for fo in range(HPC):
                        ps = pbkp.tile([128, T_TILE], F32, tag="kn")
                        for fi in range(NFO_KV):
                            nc.tensor.matmul(
                                ps[:],
                                wkn_t[:, fi, fo * 128:(fo + 1) * 128],
                                ckv_rhs[:, fi, :],
                                start=(fi == 0), stop=(fi == NFO_KV - 1))
                        nc.scalar.activation(
                            kn_sb[:, fo, tsl], ps[:],
                            mybir.ActivationFunctionType.Copy)
                    for ts in range(T_TILE // 128):
                        kb = t * 4 + ts
                        ps = pbkp.tile([128, HPC * DV], F32, tag="vps")
                        for fi in range(NFO_KV):
                            nc.tensor.matmul(
                                ps[:],
                                ckv_rhs[:, fi, ts * 128:(ts + 1) * 128],
                                wv_t[:, fi, :],
                                start=(fi == 0), stop=(fi == NFO_KV - 1))
                        nc.vector.tensor_copy(
                            vh_sb[:, :, kb, :],
                            ps[:].rearrange("p (h d) -> p h d", h=HPC))

            # ---------- phase B-q: q_b + q rope for all token tiles ----------
            NQB = HPC * QD // 128  # 6 output tiles (4 nope + 2 rope-pair)
            with nc.named_scope("proj_bq"), \
                 tc.tile_pool(name="rhq", bufs=2) as rhqp, \
                 tc.tile_pool(name="rsq", bufs=2) as rsqp, \
                 tc.tile_pool(name="psq", bufs=1, space="PSUM") as psqp, \
                 tc.tile_pool(name="evb", bufs=3) as evbp, \
                 tc.tile_pool(name="pbq", bufs=3, space="PSUM") as pbqp, \
                 tc.tile_pool(name="prt", bufs=1, space="PSUM") as prtp:
                for t in range(NT):
                    tsl = slice(t * T_TILE, (t + 1) * T_TILE)
                    qa_rhs = rhqp.tile([128, NFO_QA, T_TILE], BF16,
                                       tag="qarhs")
                    nc.sync.dma_start(qa_rhs[:], qa_all_r[:, t, :, :])
                    ssb16 = rsqp.tile([1, T_TILE], BF16, tag="ssb16")
                    nc.sync.dma_start(ssb16[:], qa_all[t, QLR:NGQ, :])
                    ssr = rsqp.tile([1, T_TILE], F32R, tag="ssr")
                    nc.vector.tensor_copy(ssr[:], ssb16[:])
                    # rstd is applied at eviction (free-dim broadcast), so
                    # the matmuls below start as soon as the rhs lands
                    rstd = rstd_tile(ssr[:], QLR, rsqp, psqp, "rq")
                    for fo in range(NQB):
                        ps = pbqp.tile([128, T_TILE], F32, tag="qb")
                        for fi in range(NFO_QA):
                            nc.tensor.matmul(
                                ps[:],
                                wqb_t[:, fi, fo * 128:(fo + 1) * 128],
                                qa_rhs[:, fi, :],
                                start=(fi == 0), stop=(fi == NFO_QA - 1))
                        if fo < HPC:  # nope (rstd folded into eviction)
                            nc.vector.tensor_tensor(
                                qn_sb[:, fo, tsl], ps[:], rstd[:],
                                mybir.AluOpType.mult)
                        else:  # rope pair: rows = heads (2j, 2j+1)
                            qpe = evbp.tile([128, T_TILE], F32R, tag="qpe")
                            nc.vector.tensor_tensor(
                                qpe[:], ps[:], rstd[:],
                                mybir.AluOpType.mult)
                            rps = prtp.tile([128, T_TILE], F32, tag="rot")
                            nc.tensor.matmul(rps[:], p128_t[:], qpe[:],
                                             start=True, stop=True)
                            tmp = evbp.tile([128, T_TILE], F32, tag="tmp")
                            nc.vector.tensor_tensor(
                                tmp[:], cos_t[:, t, :], qpe[:],
                                mybir.AluOpType.mult)
                            rot = evbp.tile([128, T_TILE], F32, tag="rot2")
                            nc.vector.tensor_tensor(
                                rot[:], sin_t[:, t, :], rps[:],
                                mybir.AluOpType.mult)
                            qro = evbp.tile([128, T_TILE], BF16, tag="qro")
                            nc.vector.tensor_tensor(
                                qro[:], tmp[:], rot[:], mybir.AluOpType.add)
                            j = fo - HPC
                            # partition shift 64->0 for the odd head needs DMA
                            nc.scalar.dma_start(
                                qr_sb[0:64, 2 * j, tsl], qro[0:64, :])
                            nc.scalar.dma_start(
                                qr_sb[0:64, 2 * j + 1, tsl], qro[64:128, :])

            # ---------- phase C: attention + fused o_proj ----------
            with nc.named_scope("attn"), \
                 tc.tile_pool(name="pt", bufs=4) as ptp, \
                 tc.tile_pool(name="ao", bufs=2) as aop, \
                 tc.tile_pool(name="oe", bufs=2) as oep, \
                 tc.tile_pool(name="sps", bufs=2, space="PSUM") as spsp, \
                 tc.tile_pool(name="avs", bufs=2, space="PSUM") as avsp, \
                 tc.tile_pool(name="lps", bufs=2, space="PSUM") as lpsp, \
                 tc.tile_pool(name="pos_", bufs=2, space="PSUM") as posp:
                for qt in range(NT):
                    qsl = slice(qt * T_TILE, (qt + 1) * T_TILE)
                    at_full = aop.tile([128, HPC, T_TILE], BF16, tag="atf")
                    nkb = 4 * qt + 4
                    for h in range(HPC):
                        av_ps = avsp.tile([128, T_TILE], F32, tag="av")
                        l_ps = lpsp.tile([128, T_TILE], F32, tag="l")
                        for kb in range(nkb):
                            sps = spsp.tile([128, T_TILE], F32, tag="s")
                            nc.tensor.matmul(
                                sps[:],
                                kn_sb[:, h, kb * 128:(kb + 1) * 128],
                                qn_sb[:, h, qsl], start=True, stop=False)
                            nc.tensor.matmul(
                                sps[:], krot_sb[:, kb * 128:(kb + 1) * 128],
                                qr_sb[:, h, qsl], start=False, stop=True)
                            pt = ptp.tile([128, T_TILE], BF16, tag="p")
                            nc.scalar.activation(
                                pt[:], sps[:],
                                mybir.ActivationFunctionType.Exp, scale=SCALE)
                            j = kb - 4 * qt
                            if j >= 0:
                                nc.vector.tensor_tensor(
                                    pt[:], pt[:], masks_t[:, j, :],
                                    mybir.AluOpType.mult)
                            nc.tensor.matmul(
                                av_ps[:], vh_sb[:, h, kb, :], pt[:],
                                start=(kb == 0), stop=(kb == nkb - 1))
                            nc.tensor.matmul(
                                l_ps[:], onesb_t[:], pt[:],
                                start=(kb == 0), stop=(kb == nkb - 1))
                        rec = ptp.tile([128, T_TILE], F32, tag="rec")
                        nc.vector.reciprocal_approx_fast(rec[:], l_ps[:])
                        nc.vector.tensor_tensor(
                            at_full[:, h, :], av_ps[:], rec[:],
                            mybir.AluOpType.mult)
                    # fused o_proj for this q-tile
                    for ts in range(T_TILE // 128):
                        tok0 = qt * T_TILE + ts * 128
                        oe = oep.tile([128, HID], F32, tag="oe")
                        for ho in range(HID // T_TILE):
                            ps = posp.tile([128, T_TILE], F32, tag="po")
                            for fs in range(HPC):
                                nc.tensor.matmul(
                                    ps[:],
                                    at_full[:, fs, ts * 128:(ts + 1) * 128],
                                    wo_sb[:, fs,
                                          ho * T_TILE:(ho + 1) * T_TILE],
                                    start=(fs == 0), stop=(fs == HPC - 1))
                            if ho % 2 == 0:
                                nc.scalar.activation(
                                    oe[:, ho * T_TILE:(ho + 1) * T_TILE],
                                    ps[:],
                                    mybir.ActivationFunctionType.Copy)
                            else:
                                nc.vector.tensor_copy(
                                    oe[:, ho * T_TILE:(ho + 1) * T_TILE],
                                    ps[:])
                        q_ = nc.sync if ts % 2 == 0 else nc.gpsimd
                        q_.dma_start(out[tok0:tok0 + 128, :], oe[:])
            res.release()
            wres.release()

    nc.compile()
    _BUILD_CACHE[key] = nc
    return nc


def _host_consts():
    ivf = (1.0 / (ROPE_BASE ** (np.arange(0, DR, 2, dtype=np.float64) / DR)))
    ivf = ivf.astype(np.float32)                       # [32]
    inv_freq128 = np.tile(ivf, 4).reshape(128, 1)

    rot = np.zeros((DR, DR), np.float32)               # rot(x) = P @ x
    for d in range(32):
        rot[d, d + 32] = -1.0
        rot[d + 32, d] = 1.0
    rotT = rot.T
    p128 = np.zeros((128, 128), np.float32)
    p128[:64, :64] = rotT
    p128[64:, 64:] = rotT

    kk = np.arange(128)[None, :, None]                 # [1,128,1]
    jj = np.arange(4)[:, None, None]                   # [4,1,1]
    qq = np.arange(T_TILE)[None, None, :]              # [1,1,512]
    masks = ((jj * 128 + kk) <= qq).astype(NP_BF16)    # [4,128,512]
    masks = np.ascontiguousarray(
        masks.transpose(1, 0, 2)).reshape(128, -1)     # [p][j][t]

    return inv_freq128, p128, masks


def _sw(a, ntile):
    """[ntile*128, F] -> partition-major [128, ntile*F] (contiguous)."""
    a = a.reshape(ntile, 128, -1).transpose(1, 0, 2)
    return np.ascontiguousarray(a).reshape(128, -1).astype(NP_BF16)


LAST_RES = None


def kernel(_debug=False, **inputs):
    hidden_states = np.asarray(inputs["hidden_states"], np.float32)
    position_ids = np.asarray(inputs["position_ids"])
    W_qa = np.asarray(inputs["W_qa"], np.float32)
    b_qa = np.asarray(inputs["b_qa"], np.float32)
    w_qa_ln = np.asarray(inputs["w_qa_ln"], np.float32)
    W_qb = np.asarray(inputs["W_qb"], np.float32)
    W_kva = np.asarray(inputs["W_kva"], np.float32)
    b_kva = np.asarray(inputs["b_kva"], np.float32)
    w_kva_ln = np.asarray(inputs["w_kva_ln"], np.float32)
    W_kvb = np.asarray(inputs["W_kvb"], np.float32)
    W_o = np.asarray(inputs["W_o"], np.float32)

    nc = build_kernel(debug=_debug)

    inv_freq128, p128, masks = _host_consts()

    # phase-A weights: [p][fo][hi][c] so each fo-tile DMA is contiguous
    w_qaT = np.ascontiguousarray(
        W_qa.reshape(NFO_QA, 128, NHI, 128).transpose(3, 0, 2, 1)
    ).reshape(128, -1).astype(NP_BF16)
    W_kva_p = np.zeros((640, HID), np.float32)
    W_kva_p[:NKV] = W_kva
    w_kvaT = np.ascontiguousarray(
        W_kva_p.reshape(5, 128, NHI, 128).transpose(3, 0, 2, 1)
    ).reshape(128, -1).astype(NP_BF16)
    # fold the rmsnorm gammas into the second-stage weights
    W_qb_h = (W_qb * w_qa_ln[None, :]).reshape(NH, QD, QLR)
    W_kvb_h = (W_kvb * w_kva_ln[None, :]).reshape(NH, DN + DV, KVLR)
    b_qa_t = np.ascontiguousarray(b_qa.reshape(NFO_QA, 128).T)
    b_kva_p = np.zeros(640, np.float32)
    b_kva_p[:NKV] = b_kva
    b_kva_t = np.ascontiguousarray(b_kva_p.reshape(5, 128).T)
    ones_sq = np.ones((128, 128), np.float32)
    ones_sqb = np.ones((128, 128), NP_BF16)

    in_maps = []
    for c in range(N_CORES):
        b = c // TPG
        g = c % TPG
        hs = list(range(g * HPC, (g + 1) * HPC))
        # q_b columns: nope blocks by head then rope blocks by head
        qb_nope = np.concatenate([W_qb_h[h, :DN, :] for h in hs], 0)
        qb_rope = np.concatenate([W_qb_h[h, DN:, :] for h in hs], 0)
        w_qbT = _sw(np.concatenate([qb_nope, qb_rope], 0).T, NFO_QA)
        w_kvb_nT = _sw(
            np.concatenate([W_kvb_h[h, :DN, :] for h in hs], 0).T, NFO_KV)
        w_kvb_vT = _sw(
            np.concatenate([W_kvb_h[h, DN:, :] for h in hs], 0).T, NFO_KV)
        w_oT = _sw(W_o[:, g * HPC * DV:(g + 1) * HPC * DV].T, HPC)
        pos_b = position_ids[b].astype(np.int32)
        in_maps.append({
            "xTl": _sw(np.ascontiguousarray(
                hidden_states[b].T[:, g * T_TILE:(g + 1) * T_TILE]), NHI),
            "w_qaT": w_qaT, "w_kvaT": w_kvaT,
            "w_qbT": w_qbT, "w_kvb_nT": w_kvb_nT, "w_kvb_vT": w_kvb_vT,
            "w_oT": w_oT,
            "b_qa": b_qa_t, "b_kva": b_kva_t,
            "pos": np.ascontiguousarray(pos_b.reshape(1, S)),
            "pos_l": np.ascontiguousarray(
                pos_b[g * T_TILE:(g + 1) * T_TILE].reshape(1, T_TILE)),
            "inv_freq": inv_freq128,
            "p128": p128, "ones_sq": ones_sq, "ones_sqb": ones_sqb,
            "masks": masks,
        })

    res = run_bass_kernel_spmd(nc, in_maps, list(range(N_CORES)))
    global LAST_RES
    LAST_RES = res

    out = np.zeros((B, S, HID), np.float32)
    for c in range(N_CORES):
        out[c // TPG] += res.results[c]["out"]
    return out


if __name__ == "__main__":
    import time
    t0 = time.time()
    build_kernel()
    print(f"build+compile: {time.time()-t0:.1f}s")
